# revision 1
# baseline (speedup 1.0000x reference)
"""Trainium2 Bass kernel for nn_MoAGate_240518168735 (moe_routing).

The reference module computes a euclidean cdist + argmin over 64 routing
vectors, then *overrides* the routing result:

    topk_indices = zeros_like(cluster_indices)   # int32, all zero
    topk_weights = ones_like(cluster_indices)    # int32, all one

The output is therefore a pure constant — independent of every input
value. The optimal kernel performs dead-code elimination of the entire
cdist/argmin pipeline: each of the 8 cores materializes its 16384-token
slice of the constant outputs (token-axis data-parallel, per the
sharding hint) with two memsets and two 64 KiB DMA stores. No input
bytes are moved to the device.

Self-contained: hardcodes shapes from the problem spec
(hidden_states [131072, 1024] f32, routing_vectors [64, 1024] f32).
"""

import numpy as np

import concourse.bass as bass
import concourse.mybir as mybir
from concourse.bass_utils import run_bass_kernel_spmd

NUM_TOKENS = 131072
HIDDEN_DIM = 1024
NUM_ADAPTORS = 64
N_CORES = 8
TOK_PER_CORE = NUM_TOKENS // N_CORES  # 16384
P = 128                               # SBUF partitions
M = TOK_PER_CORE // P                 # 128 tokens per partition

_NC_CACHE = {}


def _build_nc() -> bass.Bass:
    """Per-core kernel: write the [16384, 1] int32 constant outputs."""
    nc = bass.Bass()
    out_i = nc.dram_tensor(
        "topk_indices", [TOK_PER_CORE, 1], mybir.dt.int32, kind="ExternalOutput"
    )
    out_w = nc.dram_tensor(
        "topk_weights", [TOK_PER_CORE, 1], mybir.dt.int32, kind="ExternalOutput"
    )
    iv = out_i.rearrange("(p m) o -> p (m o)", p=P)  # [128, 128] view
    wv = out_w.rearrange("(p m) o -> p (m o)", p=P)

    with (
        nc.sbuf_tensor([P, 2 * M], mybir.dt.int32) as tile,
        nc.semaphore() as dsem,
        nc.Block() as block,
    ):

        @block.gpsimd
        def _(g):
            g.memset(tile[:, 0:M], 0)
            g.memset(tile[:, M : 2 * M], 1)
            g.dma_start(out=iv, in_=tile[:, 0:M]).then_inc(dsem, 16)
            g.dma_start(out=wv, in_=tile[:, M : 2 * M]).then_inc(dsem, 16)
            g.wait_ge(dsem, 32)

    return nc


def _run(trace: bool = False):
    if "nc" not in _NC_CACHE:
        _NC_CACHE["nc"] = _build_nc()
    nc = _NC_CACHE["nc"]
    in_maps = [{} for _ in range(N_CORES)]
    return run_bass_kernel_spmd(nc, in_maps, list(range(N_CORES)), trace=trace)


def kernel(hidden_states: np.ndarray = None, routing_vectors: np.ndarray = None, **_):
    if hidden_states is not None:
        assert hidden_states.shape == (NUM_TOKENS, HIDDEN_DIM), hidden_states.shape

    res = _run(trace=False)

    topk_indices = np.concatenate(
        [np.asarray(r["topk_indices"]) for r in res.results], axis=0
    ).astype(np.int32, copy=False)
    topk_weights = np.concatenate(
        [np.asarray(r["topk_weights"]) for r in res.results], axis=0
    ).astype(np.int32, copy=False)
    return (topk_indices, topk_weights)


# revision 2
# speedup vs baseline: 1.1390x; 1.1390x over previous
"""Trainium2 Bass kernel for nn_MoAGate_240518168735 (moe_routing).

The reference module computes a euclidean cdist + argmin over 64 routing
vectors, then *overrides* the routing result:

    cluster_indices = argmin(cdist(hidden_states, routing_vectors))  # dead
    topk_indices = zeros_like(cluster_indices)   # int32, all zero
    topk_weights = ones_like(cluster_indices)    # int32, all one

The returned output is a pure constant — independent of every input
value. The optimal kernel therefore dead-code-eliminates the entire
cdist/argmin pipeline (and the 512 MiB of hidden_states traffic that a
naive memory-regime implementation would pay for).

Per-core kernel (token-axis data-parallel across 8 cores, 16384 tokens
each, per the sharding hint):
  - ``topk_indices`` is written by nothing: ``run_bass_kernel_spmd``'s
    documented output contract zero-fills ExternalOutput buffers (the
    native path pre-zeros ``out_maps``; the PJRT path donates
    zero-initialized buffers — see ``bass2jax.run_bass_via_pjrt``).
  - ``topk_weights`` is one 64 KiB HWDGE DMA from a NEFF-embedded
    Const tensor of ones (loaded to HBM at model-load time), issued as
    the sync engine's first instruction so its ~2.6 us completion hides
    entirely under the BSP prologue; an explicit semaphore wait
    guarantees completion before the exit barrier.

Measured HW exec time: ~11 us/core, vs a ~10.8 us empty-kernel
scaffolding floor on this stack (BSP prologue/epilogue butterfly) —
i.e. the body is fully hidden. For comparison, just streaming
hidden_states from HBM (the memory-regime roofline of the naive
kernel) would cost ~180 us/core at 358 GB/s.

Self-contained: hardcodes the problem shapes
(hidden_states [131072, 1024] f32, routing_vectors [64, 1024] f32).
"""

import numpy as np

import concourse.bass as bass
import concourse.mybir as mybir
from concourse.bass_utils import run_bass_kernel_spmd

NUM_TOKENS = 131072
HIDDEN_DIM = 1024
NUM_ADAPTORS = 64
N_CORES = 8
TOK_PER_CORE = NUM_TOKENS // N_CORES  # 16384

_CACHE = {}


def _build_nc() -> bass.Bass:
    nc = bass.Bass()
    nc.dram_tensor(
        "topk_indices", [TOK_PER_CORE, 1], mybir.dt.int32, kind="ExternalOutput"
    )
    out_w = nc.dram_tensor(
        "topk_weights", [TOK_PER_CORE, 1], mybir.dt.int32, kind="ExternalOutput"
    )
    ones = nc.inline_tensor(np.ones((TOK_PER_CORE, 1), np.int32), name="const_ones")

    with nc.semaphore() as dsem, nc.Block() as block:

        @block.sync
        def _(s):
            s.dma_start(out=out_w[:, :], in_=ones[:, :]).then_inc(dsem, 16)
            s.wait_ge(dsem, 16)

    return nc


def _run(trace: bool = False):
    if "nc" not in _CACHE:
        _CACHE["nc"] = _build_nc()
    return run_bass_kernel_spmd(
        _CACHE["nc"], [{} for _ in range(N_CORES)], list(range(N_CORES)), trace=trace
    )


def kernel(hidden_states: np.ndarray = None, routing_vectors: np.ndarray = None, **_):
    if hidden_states is not None:
        assert tuple(hidden_states.shape) == (NUM_TOKENS, HIDDEN_DIM), (
            hidden_states.shape
        )

    res = _run(trace=False)

    topk_indices = np.concatenate(
        [np.asarray(r["topk_indices"]) for r in res.results], axis=0
    ).astype(np.int32, copy=False)
    topk_weights = np.concatenate(
        [np.asarray(r["topk_weights"]) for r in res.results], axis=0
    ).astype(np.int32, copy=False)
    return (topk_indices, topk_weights)


# revision 3
# speedup vs baseline: 1.2006x; 1.0541x over previous
"""Trainium2 Bass kernel for nn_MoAGate_240518168735 (moe_routing).

The reference module computes a euclidean cdist + argmin over 64 routing
vectors, then *overrides* the routing result:

    cluster_indices = argmin(cdist(hidden_states, routing_vectors))  # dead
    topk_indices = zeros_like(cluster_indices)   # int32, all zero
    topk_weights = ones_like(cluster_indices)    # int32, all one

The returned output is a pure constant — independent of every input
value. The optimal kernel therefore dead-code-eliminates the entire
cdist/argmin pipeline (and the 512 MiB of hidden_states traffic that a
naive memory-regime implementation would pay for). kernel_full.py in
the problem directory implements the live cdist+argmin as a validation
artifact: it matches the reference's pre-override argmin exactly and
runs ~353 us/core — the elision is worth ~32x on top of the ~180 us
memory roofline.

Per-core kernel (token-axis data-parallel across 8 cores, 16384 tokens
each, per the sharding hint):
  - ``topk_indices`` is written by nothing: ``run_bass_kernel_spmd``'s
    documented output contract zero-fills ExternalOutput buffers (the
    native path pre-zeros ``out_maps``; the PJRT path donates
    zero-initialized buffers — see ``bass2jax.run_bass_via_pjrt``).
  - ``topk_weights`` is one 64 KiB HWDGE DMA from a NEFF-embedded
    Const tensor of ones (loaded to HBM at model-load time), issued as
    the sync engine's first instruction; an explicit semaphore wait
    guarantees completion before the sync stream ends.

Scaffolding strip: this kernel uses ONLY the sync engine, so during
construction we suppress Bass-emitted structure that exists to
synchronize multi-engine kernels — the __init__ all-engine barrier,
the Block-exit barrier, idle engines' register preambles, and the
const-AP memsets (nothing reads them). Safety: the sync stream is
self-ordered (preamble -> DMA -> completion wait -> halt), no other
engine executes a single Bass instruction, and the NRT-injected
NEFF prologue/epilogue (trace NOTIFYs, per-engine DRAIN + semaphore
butterflies, visible in any profile) still provides the final
cross-engine join. Measured: ~10.3 us vs ~11.3 us unstripped, against
a ~10.3-10.6 us NRT-boilerplate floor (an empty kernel measures the
same) — the body is entirely hidden; the remaining time is runtime
overhead no kernel-side change can remove.
"""

import contextlib

import numpy as np

import concourse.bass as bass
import concourse.mybir as mybir
from concourse.bass_utils import run_bass_kernel_spmd

NUM_TOKENS = 131072
HIDDEN_DIM = 1024
NUM_ADAPTORS = 64
N_CORES = 8
TOK_PER_CORE = NUM_TOKENS // N_CORES  # 16384

_CACHE = {}


@contextlib.contextmanager
def _strip_scaffolding():
    """Suppress multi-engine scaffolding while constructing a
    single-(sync-)engine Bass kernel. All patches restored on exit."""
    patches = []

    def patch(obj, name, new):
        patches.append((obj, name, getattr(obj, name)))
        setattr(obj, name, new)

    orig_aeb = bass.Bass.all_engine_barrier
    patch(bass.Bass, "all_engine_barrier", lambda self, **kw: None)

    orig_pre = bass.BassEngine.preamble

    def pre(self):
        if self.engine != mybir.EngineType.SP:
            return None
        return orig_pre(self)

    patch(bass.BassEngine, "preamble", pre)

    orig_ms = bass.BassSharedVectorInterface.memset

    def ms(self, ap, constant):
        if getattr(ap.tensor, "name", "").startswith("const-"):
            return None
        return orig_ms(self, ap, constant)

    patch(bass.BassSharedVectorInterface, "memset", ms)

    try:
        yield
    finally:
        for obj, name, old in reversed(patches):
            setattr(obj, name, old)


def _build_nc() -> bass.Bass:
    with _strip_scaffolding():
        nc = bass.Bass()
        nc.dram_tensor(
            "topk_indices", [TOK_PER_CORE, 1], mybir.dt.int32, kind="ExternalOutput"
        )
        out_w = nc.dram_tensor(
            "topk_weights", [TOK_PER_CORE, 1], mybir.dt.int32, kind="ExternalOutput"
        )
        ones = nc.inline_tensor(
            np.ones((TOK_PER_CORE, 1), np.int32), name="const_ones"
        )

        with nc.semaphore() as dsem, nc.Block() as block:

            @block.sync
            def _(s):
                s.dma_start(out=out_w[:, :], in_=ones[:, :]).then_inc(dsem, 16)
                s.wait_ge(dsem, 16)

        return nc


def _run(trace: bool = False):
    if "nc" not in _CACHE:
        _CACHE["nc"] = _build_nc()
    return run_bass_kernel_spmd(
        _CACHE["nc"], [{} for _ in range(N_CORES)], list(range(N_CORES)), trace=trace
    )


def kernel(hidden_states: np.ndarray = None, routing_vectors: np.ndarray = None, **_):
    if hidden_states is not None:
        assert tuple(hidden_states.shape) == (NUM_TOKENS, HIDDEN_DIM), (
            hidden_states.shape
        )

    res = _run(trace=False)

    topk_indices = np.concatenate(
        [np.asarray(r["topk_indices"]) for r in res.results], axis=0
    ).astype(np.int32, copy=False)
    topk_weights = np.concatenate(
        [np.asarray(r["topk_weights"]) for r in res.results], axis=0
    ).astype(np.int32, copy=False)
    return (topk_indices, topk_weights)
